# revision 4
# baseline (speedup 1.0000x reference)
import sys
sys.path.insert(0, "/opt/trn_rl_repo")
import numpy as np

N = 100000
E = 800000
D = 128
P = 8
NSH = 12500          # nodes per core
NSH_PAD = 12544      # 98 * 128
ALPHA = 0.5
CHUNK = 32767        # real rows per gather chunk (slot 32767 of each chunk is zero)
NCHUNK = 4
CALL = 896           # idxs per call (56/113 descs per ring: ring-safe)
CALL_PAD = 896
SERIALIZE = True     # dummy-reader between same-buffer scatters: required —
                     # also throttles SWDGE ring occupancy (no-reader variant
                     # intermittently overflows the 128-desc ring and wedges
                     # the device)


def _plan_direction(gather_nodes, seg_nodes, core):
    """Duplicate-free call plan: within one call every scatter target is
    unique (edges grouped by round-robin rank within their segment), so no
    same-row RMW races inside a call. Cross-call ordering is enforced in the
    device program. Returns list of (chunk_id, gidx[CALL], sidx[CALL]).
    """
    chunk = gather_nodes // CHUNK
    local = gather_nodes % CHUNK
    calls = []
    for c in range(NCHUNK):
        m = chunk == c
        loc, seg = local[m], seg_nodes[m]
        order = np.argsort(seg, kind="stable")
        loc, seg = loc[order], seg[order]
        if seg.size:
            first = np.r_[True, seg[1:] != seg[:-1]]
            idx_of_first = np.maximum.accumulate(
                np.where(first, np.arange(seg.size), 0))
            rank = np.arange(seg.size) - idx_of_first
            nrank = int(rank.max()) + 1
        else:
            rank, nrank = np.zeros(0, np.int64), 0
        for j in range(nrank):
            mj = rank == j
            lj, sj = loc[mj], seg[mj]
            for s0 in range(0, lj.size, CALL):
                lj_k = lj[s0:s0 + CALL]
                sj_k = sj[s0:s0 + CALL]
                g = np.full(CALL, CHUNK, np.int16)   # pad: gather zero row
                s = np.zeros(CALL, np.int16)          # pad: add zeros to 0
                g[:lj_k.size] = lj_k.astype(np.int16)
                s[:sj_k.size] = sj_k.astype(np.int16)
                calls.append((c, g, s))
    return calls


def _pad_calls(calls, ncalls):
    """Pad call list to ncalls uniform calls of CALL idxs (zero-row pads)."""
    out = []
    for c, lj, sj in calls:
        g = np.full(CALL, CHUNK, np.int16)       # zero-row of the chunk
        s = np.zeros(CALL, np.int16)             # add zeros to node 0
        g[:lj.size] = lj.astype(np.int16)
        s[:sj.size] = sj.astype(np.int16)
        out.append((c, g, s))
    while len(out) < ncalls:
        out.append((0, np.full(CALL, CHUNK, np.int16), np.zeros(CALL, np.int16)))
    return out


def _wrap16(a):
    # idx i -> [i%16, i//16], replicated to 128 partitions
    return np.tile(a.reshape(-1, 16).T, (8, 1))


def kernel(x, W_self, b_self, W_s2d, b_s2d, W_d2s, b_d2s, edge_index):
    import time as _time
    _t = [_time.perf_counter()]
    def _lap(tag):
        now = _time.perf_counter()
        print(f"[kernel-timing] {tag}: {now - _t[0]:.3f}s", file=sys.stderr)
        _t[0] = now
    x = np.asarray(x, np.float32)
    W_self = np.asarray(W_self, np.float32)
    b_self = np.asarray(b_self, np.float32)
    W_s2d = np.asarray(W_s2d, np.float32)
    b_s2d = np.asarray(b_s2d, np.float32)
    W_d2s = np.asarray(W_d2s, np.float32)
    b_d2s = np.asarray(b_d2s, np.float32)
    src = np.asarray(edge_index[0], np.int64)
    dst = np.asarray(edge_index[1], np.int64)

    # x_aug: 4 chunks of 32768 rows; chunk c rows [32768c..] = x[32767c .. 32767c+32766], last slot zero
    x_aug = np.zeros((NCHUNK * 32768, D), np.float32)
    for c in range(NCHUNK):
        lo = c * CHUNK
        hi = min(lo + CHUNK, N)
        x_aug[c * 32768: c * 32768 + (hi - lo)] = x[lo:hi]

    x_T = np.zeros((D, P * NSH_PAD), np.float32)
    x_T[:, :N] = 0.0
    # per-core padded transposed x of own shard
    for core in range(P):
        xo = x[core * NSH:(core + 1) * NSH]
        x_T[:, core * NSH_PAD: core * NSH_PAD + NSH] = xo.T

    deg_in = np.bincount(dst, minlength=N).astype(np.float32)
    deg_out = np.bincount(src, minlength=N).astype(np.float32)
    inv_in = 1.0 / np.maximum(deg_in, 1.0)
    inv_out = 1.0 / np.maximum(deg_out, 1.0)

    W1 = (1.0 - ALPHA) * W_s2d
    W2 = ALPHA * W_d2s
    b_tot = b_self + (1.0 - ALPHA) * b_s2d + ALPHA * b_d2s

    _lap("host-prep-arrays")
    # --- per-core plans ---
    plans = []
    for core in range(P):
        lo, hi = core * NSH, (core + 1) * NSH
        m_in = (dst >= lo) & (dst < hi)
        calls_in = _plan_direction(src[m_in], dst[m_in] - lo, core)
        m_out = (src >= lo) & (src < hi)
        calls_out = _plan_direction(dst[m_out], src[m_out] - lo, core)
        plans.append((calls_in, calls_out))
    # uniform schedule: per chunk, #calls = max over cores
    def make_sched(idx):
        counts = np.zeros((P, NCHUNK), np.int64)
        for core in range(P):
            for c, _, _ in plans[core][idx]:
                counts[core, c] += 1
        per_chunk = counts.max(axis=0)
        sched = []
        for c in range(NCHUNK):
            sched += [c] * int(per_chunk[c])
        return sched, per_chunk

    sched_in, pc_in = make_sched(0)
    sched_out, pc_out = make_sched(1)

    def conform(calls, per_chunk):
        # place each core's chunk-c calls into that chunk's slot block
        out = []
        for c in range(NCHUNK):
            mine = [t for t in calls if t[0] == c]
            out.extend(mine)
            for _ in range(int(per_chunk[c]) - len(mine)):
                out.append((c, np.full(CALL, CHUNK, np.int16),
                            np.zeros(CALL, np.int16)))
        return out

    padded = []
    for core in range(P):
        padded.append((conform(plans[core][0], pc_in),
                       conform(plans[core][1], pc_out)))

    # --- build per-core input arrays ---
    def build_idx(calls):
        g = np.concatenate([_wrap16(c[1]) for c in calls], axis=1)
        s = np.concatenate([_wrap16(c[2]) for c in calls], axis=1)
        return g.astype(np.int16), s.astype(np.int16)

    in_maps = []
    for core in range(P):
        gi, si = build_idx(padded[core][0])
        go, so = build_idx(padded[core][1])
        lo = core * NSH
        invi = np.zeros(NSH_PAD, np.float32)
        invo = np.zeros(NSH_PAD, np.float32)
        invi[:NSH] = inv_in[lo:lo + NSH]
        invo[:NSH] = inv_out[lo:lo + NSH]
        in_maps.append({
            "x_aug": x_aug,
            "x_ownT": np.ascontiguousarray(
                x_T[:, core * NSH_PAD:(core + 1) * NSH_PAD]),
            "gidx_in": gi, "sidx_in": si,
            "gidx_out": go, "sidx_out": so,
            "inv_in": invi.reshape(98, 128).T.copy(),
            "inv_out": invo.reshape(98, 128).T.copy(),
            "W_self": W_self, "W1": W1, "W2": W2,
            "b": b_tot.reshape(128, 1),
            "zeros": np.zeros((NSH_PAD, D), np.float32),
            "ident": np.eye(128, dtype=np.float32),
        })

    _lap("host-plan+inmaps")
    nc = _build_program(sched_in, sched_out)
    _lap("bass-compile")
    from concourse.bass_utils import run_bass_kernel_spmd
    res = run_bass_kernel_spmd(nc, in_maps, list(range(P)))
    _lap("device-run")
    out = np.empty((N, D), np.float32)
    for core in range(P):
        out[core * NSH:(core + 1) * NSH] = res.results[core]["out"][:NSH]
    return out


def _build_program(sched_in, sched_out, niter=1):
    from concourse import bacc, tile, mybir, library_config
    import concourse.bass as bass

    f32 = mybir.dt.float32
    i16 = mybir.dt.int16
    nc = bacc.Bacc("TRN2", target_bir_lowering=False, debug=False,
                   num_swdge_queues=3)

    ncalls_in, ncalls_out = len(sched_in), len(sched_out)
    xa = nc.dram_tensor("x_aug", [NCHUNK * 32768, D], f32, kind="ExternalInput")
    xT = nc.dram_tensor("x_ownT", [D, NSH_PAD], f32, kind="ExternalInput")
    gii = nc.dram_tensor("gidx_in", [128, ncalls_in * CALL // 16], i16, kind="ExternalInput")
    sii = nc.dram_tensor("sidx_in", [128, ncalls_in * CALL // 16], i16, kind="ExternalInput")
    gio = nc.dram_tensor("gidx_out", [128, ncalls_out * CALL // 16], i16, kind="ExternalInput")
    sio = nc.dram_tensor("sidx_out", [128, ncalls_out * CALL // 16], i16, kind="ExternalInput")
    ivi = nc.dram_tensor("inv_in", [128, 98], f32, kind="ExternalInput")
    ivo = nc.dram_tensor("inv_out", [128, 98], f32, kind="ExternalInput")
    Ws = nc.dram_tensor("W_self", [D, D], f32, kind="ExternalInput")
    W1 = nc.dram_tensor("W1", [D, D], f32, kind="ExternalInput")
    W2 = nc.dram_tensor("W2", [D, D], f32, kind="ExternalInput")
    bt = nc.dram_tensor("b", [D, 1], f32, kind="ExternalInput")
    zz = nc.dram_tensor("zeros", [NSH_PAD, D], f32, kind="ExternalInput")
    idn = nc.dram_tensor("ident", [D, D], f32, kind="ExternalInput")
    agg_in = nc.dram_tensor("agg_in", [NSH_PAD, D], f32)
    agg_out = nc.dram_tensor("agg_out", [NSH_PAD, D], f32)
    out = nc.dram_tensor("out", [NSH_PAD, D], f32, kind="ExternalOutput")

    COLS = CALL // 16  # idx cols per call

    with tile.TileContext(nc) as tc:
        nc.gpsimd.load_library(library_config.mlp)
        with tc.tile_pool(name="const", bufs=1) as cp, \
             tc.tile_pool(name="gt", bufs=2) as gp, \
             tc.tile_pool(name="ep", bufs=3) as ep, \
             tc.tile_pool(name="ps", bufs=2, space="PSUM") as pp:
            # constants
            gii_s = cp.tile([128, ncalls_in * COLS], i16)
            sii_s = cp.tile([128, ncalls_in * COLS], i16)
            gio_s = cp.tile([128, ncalls_out * COLS], i16)
            sio_s = cp.tile([128, ncalls_out * COLS], i16)
            nc.sync.dma_start(gii_s[:], gii[:])
            nc.sync.dma_start(sii_s[:], sii[:])
            nc.sync.dma_start(gio_s[:], gio[:])
            nc.sync.dma_start(sio_s[:], sio[:])
            ivi_s = cp.tile([128, 98], f32)
            ivo_s = cp.tile([128, 98], f32)
            nc.sync.dma_start(ivi_s[:], ivi[:])
            nc.sync.dma_start(ivo_s[:], ivo[:])
            Ws_s = cp.tile([D, D], f32)
            W1_s = cp.tile([D, D], f32)
            W2_s = cp.tile([D, D], f32)
            b_s = cp.tile([D, 1], f32)
            id_s = cp.tile([D, D], f32)
            nc.sync.dma_start(Ws_s[:], Ws[:])
            nc.sync.dma_start(W1_s[:], W1[:])
            nc.sync.dma_start(W2_s[:], W2[:])
            nc.sync.dma_start(b_s[:], bt[:])
            nc.sync.dma_start(id_s[:], idn[:])

            # zero agg buffers
            nc.sync.dma_start(agg_in[:], zz[:])
            nc.sync.dma_start(agg_out[:], zz[:])

            # gather + scatter chains; a dummy reader of the agg buffer
            # between consecutive same-buffer scatters forces each scatter's
            # DMA to complete before the next starts (cross-call same-row
            # RMWs on different DMA engines would otherwise race).
            def emit_call(k, c, gidx_s, sidx_s, agg, dirtag, serialize=SERIALIZE):
                t = gp.tile([128, CALL_PAD // 128, D], f32, tag="gath")
                nc.gpsimd.dma_gather(
                    t[:], xa[c * 32768:(c + 1) * 32768, :],
                    gidx_s[:, k * COLS:(k + 1) * COLS],
                    CALL, CALL, D, queue_num=0)
                if serialize:
                    dr = gp.tile([1, 64], f32, tag=f"dummy{dirtag}")
                    nc.sync.dma_start(dr[:], agg[0:1, 0:64])
                # separate queue per direction: the two chains touch disjoint
                # buffers, so their scatter DMAs can overlap on distinct rings
                nc.gpsimd.dma_scatter_add(
                    agg[:], t[:],
                    sidx_s[:, k * COLS:(k + 1) * COLS],
                    CALL, CALL, D, queue_num=1 if dirtag == "i" else 2)

            # interleave the 8 (direction, chunk) streams round-robin so that
            # consecutive same-buffer scatters sharing a dst row are far
            # apart in time (adjacent same-stream calls are dst-disjoint
            # splits of one rank group or consecutive ranks)
            streams = []
            for c in range(NCHUNK):
                ks = [k for k, cc in enumerate(sched_in) if cc == c]
                streams.append(("i", ks))
                ks = [k for k, cc in enumerate(sched_out) if cc == c]
                streams.append(("o", ks))
            maxlen = max(len(s[1]) for s in streams)
            for r in range(maxlen):
                for d, ks in streams:
                    if r < len(ks):
                        k = ks[r]
                        if d == "i":
                            emit_call(k, sched_in[k], gii_s, sii_s, agg_in, "i")
                        else:
                            emit_call(k, sched_out[k], gio_s, sio_s, agg_out, "o")

            # epilogue per 128-node tile
            for t in range(98):
                ai = ep.tile([128, D], f32, tag="ai")
                ao = ep.tile([128, D], f32, tag="ao")
                nc.sync.dma_start(ai[:], agg_in[t * 128:(t + 1) * 128, :])
                nc.sync.dma_start(ao[:], agg_out[t * 128:(t + 1) * 128, :])
                # scale by inv degree (per-partition scalar)
                nc.vector.tensor_scalar(ai[:], ai[:], ivi_s[:, t:t + 1], None,
                                        mybir.AluOpType.mult)
                nc.vector.tensor_scalar(ao[:], ao[:], ivo_s[:, t:t + 1], None,
                                        mybir.AluOpType.mult)
                # transpose both
                pt = pp.tile([128, D], f32, tag="pt")
                nc.tensor.matmul(pt[:], ai[:], id_s[:], start=True, stop=True,
                                 is_transpose=True)
                aiT = ep.tile([128, D], f32, tag="aiT")
                nc.vector.tensor_copy(aiT[:], pt[:])
                pt2 = pp.tile([128, D], f32, tag="pt")
                nc.tensor.matmul(pt2[:], ao[:], id_s[:], start=True, stop=True,
                                 is_transpose=True)
                aoT = ep.tile([128, D], f32, tag="aoT")
                nc.vector.tensor_copy(aoT[:], pt2[:])
                # x_ownT tile direct from DRAM
                xt_t = ep.tile([128, 128], f32, tag="xt")
                nc.sync.dma_start(xt_t[:], xT[:, t * 128:(t + 1) * 128])
                # y = W_self.T @ xT + W1.T @ aiT + W2.T @ aoT   [feat_out, nodes]
                y = pp.tile([128, 128], f32, tag="y")
                nc.tensor.matmul(y[:], Ws_s[:], xt_t[:], start=True, stop=False)
                nc.tensor.matmul(y[:], W1_s[:], aiT[:], start=False, stop=False)
                nc.tensor.matmul(y[:], W2_s[:], aoT[:], start=False, stop=True)
                ysb = ep.tile([128, 128], f32, tag="ysb")
                nc.vector.tensor_scalar(ysb[:], y[:], b_s[:, 0:1], None,
                                        mybir.AluOpType.add)
                # transpose back to [nodes, feat]
                po = pp.tile([128, 128], f32, tag="po")
                nc.tensor.matmul(po[:], ysb[:], id_s[:], start=True, stop=True,
                                 is_transpose=True)
                osb = ep.tile([128, 128], f32, tag="osb")
                nc.vector.tensor_copy(osb[:], po[:])
                nc.sync.dma_start(out[t * 128:(t + 1) * 128, :], osb[:])

    nc.compile()
    return nc



# revision 14
# speedup vs baseline: 16.0625x; 16.0625x over previous
import sys
sys.path.insert(0, "/opt/trn_rl_repo")
import time as _time
import numpy as np

N = 100000
E = 800000
D = 128
P = 8
NSH = 12500          # nodes per core
NSH_PAD = 12544      # 98 * 128
NFULL = P * NSH_PAD  # 100352 rows in allgathered x
ALPHA = 0.5
CALL = 896           # idxs per call (56 idx cols per call: ring-safe)
COLS = CALL // 16
# gather chunks over x_full rows (int16 idx limit 32767): chunk c = rows
# [32768c, 32768c+32768); chunk 3 is short (2048 rows)
NCHUNK = 4
CHUNK_ROWS = (32768, 32768, 32768, NFULL - 3 * 32768)
# per-direction per-chunk call capacity (fixed program shape; ~20% margin
# over the expected rank-grouped call count for E/P random edges)
CAPS = (50, 50, 50, 8)
NC_DIR = sum(CAPS)          # calls per direction
CHUNK_BASE = (0, CAPS[0], CAPS[0] + CAPS[1], CAPS[0] + CAPS[1] + CAPS[2])

_verbose = True


def _log(tag, t0):
    if _verbose:
        now = _time.perf_counter()
        print(f"[kernel-timing] {tag}: {now - t0:.3f}s", file=sys.stderr)
    return _time.perf_counter()


# ---------------------------------------------------------------- planner ---

def _plan_direction(gat, seg, caps):
    """Vectorized duplicate-free call plan for one direction.

    gat: global gather node per edge; seg: global segment (scatter) node.
    Returns (gflat, sflat) int16 arrays [P, NC_DIR*CALL] filled with -1 pads,
    plus per-(core,chunk) needed call counts [P, NCHUNK].
    Within one call every scatter target is unique (edges grouped by
    round-robin rank within their (chunk, segment)), so no same-row RMW
    races inside a call.
    """
    core = seg // NSH
    loc = (seg - core * NSH).astype(np.int64)
    q = gat // NSH
    row = q * NSH_PAD + (gat - q * NSH)
    chunk = row >> 15
    lidx = row & 32767

    cc = core * NCHUNK + chunk                 # 0..31
    # sort by (core, chunk, seg); rank = run position within equal seg
    k1 = cc * (1 << 17) + seg
    o1 = np.argsort(k1, kind="stable")
    k1s = k1[o1]
    first = np.empty(E, bool)
    first[0] = True
    np.not_equal(k1s[1:], k1s[:-1], out=first[1:])
    ar = np.arange(E, dtype=np.int64)
    idx_first = np.maximum.accumulate(np.where(first, ar, 0))
    rank = ar - idx_first
    # sort by (core, chunk, rank) stable -> final edge order
    k2 = cc[o1] * E + rank
    o2 = np.argsort(k2, kind="stable")
    k2s = k2[o2]
    of = o1[o2]
    # position within each (core, chunk, rank) run
    first2 = np.empty(E, bool)
    first2[0] = True
    np.not_equal(k2s[1:], k2s[:-1], out=first2[1:])
    idx_first2 = np.maximum.accumulate(np.where(first2, ar, 0))
    posr = ar - idx_first2
    call_in_run = posr // CALL
    slot = posr - call_in_run * CALL
    # per-run call counts -> per-(core,chunk) cumulative call base per rank
    run_starts = np.flatnonzero(first2)
    run_lens = np.diff(np.r_[run_starts, E])
    run_calls = (run_lens + CALL - 1) // CALL
    run_cc = cc[of[run_starts]]
    # cumulative calls of earlier ranks within same (core,chunk)
    csum = np.cumsum(run_calls) - run_calls
    cc_first_run = np.empty(run_cc.size, bool)
    cc_first_run[0] = True
    np.not_equal(run_cc[1:], run_cc[:-1], out=cc_first_run[1:])
    arr_r = np.arange(run_cc.size)
    idx_first_cc = np.maximum.accumulate(np.where(cc_first_run, arr_r, 0))
    run_base = csum - csum[idx_first_cc]
    needed = np.zeros((P, NCHUNK), np.int64)
    last_of_cc = np.r_[cc_first_run[1:], True]
    needed[run_cc[last_of_cc] // NCHUNK, run_cc[last_of_cc] % NCHUNK] = \
        (run_base + run_calls)[last_of_cc]
    if np.any(needed > np.asarray(caps)[None, :]):
        return None, None, needed
    # flat destination index
    call_idx = run_base[np.cumsum(first2) - 1] + call_in_run
    ch_base = np.asarray(
        [0, caps[0], caps[0] + caps[1], caps[0] + caps[1] + caps[2]])
    nc_dir = int(sum(caps))
    core_f = core[of]
    chunk_f = chunk[of]
    flat = ((core_f * nc_dir + ch_base[chunk_f] + call_idx) * CALL + slot)
    # pads gather row 0 of their chunk and scatter into agg row NSH (a
    # discarded pad row); same-row pad adds may race but are never read
    gflat = np.zeros(P * nc_dir * CALL, np.int16)
    sflat = np.full(P * nc_dir * CALL, NSH, np.int16)
    gflat[flat] = lidx[of].astype(np.int16)
    sflat[flat] = loc[of].astype(np.int16)
    return (gflat.reshape(P, nc_dir * CALL),
            sflat.reshape(P, nc_dir * CALL), needed)


def _wrap16(a):
    # idx i -> [i % 16, i // 16]
    return np.ascontiguousarray(a.reshape(-1, 16).T)


# ------------------------------------------------------------- device prog --

def _build_program(caps):
    from concourse import bacc, tile, mybir, library_config

    f32 = mybir.dt.float32
    f16 = mybir.dt.float16
    i16 = mybir.dt.int16
    nc = bacc.Bacc("TRN2", target_bir_lowering=False, debug=False,
                   num_swdge_queues=3, num_devices=P)

    nc_dir = int(sum(caps))
    xs = nc.dram_tensor("x_sh", [NSH_PAD, D], f16, kind="ExternalInput")
    gii = nc.dram_tensor("gidx_in", [16, nc_dir * COLS], i16, kind="ExternalInput")
    sii = nc.dram_tensor("sidx_in", [16, nc_dir * COLS], i16, kind="ExternalInput")
    gio = nc.dram_tensor("gidx_out", [16, nc_dir * COLS], i16, kind="ExternalInput")
    sio = nc.dram_tensor("sidx_out", [16, nc_dir * COLS], i16, kind="ExternalInput")
    ivi = nc.dram_tensor("inv_in", [128, 98], f32, kind="ExternalInput")
    ivo = nc.dram_tensor("inv_out", [128, 98], f32, kind="ExternalInput")
    Ws = nc.dram_tensor("W_self", [D, D], f32, kind="ExternalInput")
    W1 = nc.dram_tensor("W1", [D, D], f32, kind="ExternalInput")
    W2 = nc.dram_tensor("W2", [D, D], f32, kind="ExternalInput")
    bt = nc.dram_tensor("b", [D, 1], f32, kind="ExternalInput")
    idn = nc.dram_tensor("ident", [D, D], f32, kind="ExternalInput")
    x_bounce = nc.dram_tensor("x_bounce", [NSH_PAD, D], f16)
    # NOTE: addr_space="Shared" for the AllGather output desyncs the mesh
    # under the axon PJRT path; plain DRAM works (slower CC but tiny anyway).
    x_full = nc.dram_tensor("x_full", [NFULL, D], f16)
    agg_in = nc.dram_tensor("agg_in", [NSH_PAD, D], f32)
    agg_out = nc.dram_tensor("agg_out", [NSH_PAD, D], f32)
    out = nc.dram_tensor("out", [NSH_PAD, D], f16, kind="ExternalOutput")

    with tile.TileContext(nc) as tc:
        nc.gpsimd.load_library(library_config.mlp)
        with tc.tile_pool(name="const", bufs=1) as cp, \
             tc.tile_pool(name="gt", bufs=2) as gp, \
             tc.tile_pool(name="ep", bufs=3) as ep, \
             tc.tile_pool(name="ps", bufs=2, space="PSUM") as pp:
            # shard -> bounce -> allgather into x_full
            nc.sync.dma_start(x_bounce[:], xs[:])
            nc.gpsimd.collective_compute(
                "AllGather", mybir.AluOpType.bypass,
                replica_groups=[list(range(P))],
                ins=[x_bounce.ap().opt()],
                outs=[x_full.ap().opt()],
            )

            # index tiles: load 16 rows, replicate to 128 partitions on device
            idx_tiles = {}
            for nm, src in (("gi", gii), ("si", sii), ("go", gio), ("so", sio)):
                t = cp.tile([128, nc_dir * COLS], i16, tag=f"idx_{nm}")
                nc.sync.dma_start(t[0:16, :], src[:])
                nc.sync.dma_start(t[16:32, :], src[:])
                nc.sync.dma_start(t[32:64, :], t[0:32, :])
                nc.sync.dma_start(t[64:128, :], t[0:64, :])
                idx_tiles[nm] = t
            ivi_s = cp.tile([128, 98], f32)
            ivo_s = cp.tile([128, 98], f32)
            nc.sync.dma_start(ivi_s[:], ivi[:])
            nc.sync.dma_start(ivo_s[:], ivo[:])
            Ws_s = cp.tile([D, D], f32)
            W1_s = cp.tile([D, D], f32)
            W2_s = cp.tile([D, D], f32)
            b_s = cp.tile([D, 1], f32)
            id_s = cp.tile([D, D], f32)
            nc.sync.dma_start(Ws_s[:], Ws[:])
            nc.sync.dma_start(W1_s[:], W1[:])
            nc.sync.dma_start(W2_s[:], W2[:])
            nc.sync.dma_start(b_s[:], bt[:])
            nc.sync.dma_start(id_s[:], idn[:])

            # zero agg buffers from an SBUF zero tile
            zt = cp.tile([128, NSH_PAD], f32)
            nc.vector.memset(zt[:], 0.0)
            for t in range(98):
                nc.sync.dma_start(agg_in[t * 128:(t + 1) * 128, :],
                                  zt[:, t * 128:(t + 1) * 128])
                nc.sync.dma_start(agg_out[t * 128:(t + 1) * 128, :],
                                  zt[:, t * 128:(t + 1) * 128])

            # gather + convert + scatter chains; a dummy reader of the agg
            # buffer between consecutive same-buffer scatters forces each
            # scatter's DMA to complete before the next starts (cross-call
            # same-row RMWs on different DMA engines would otherwise race).
            def emit_call(k, c, gkey, skey, agg, dirtag):
                t16 = gp.tile([128, CALL // 128, D], f16, tag="g16")
                if c < 3:
                    src = x_full[c * 32768:(c + 1) * 32768, :]
                else:
                    src = x_full[3 * 32768:NFULL, :]
                nc.gpsimd.dma_gather(
                    t16[:], src,
                    idx_tiles[gkey][:, k * COLS:(k + 1) * COLS],
                    CALL, CALL, D, queue_num=0)
                t32 = gp.tile([128, CALL // 128, D], f32, tag="g32")
                nc.vector.tensor_copy(t32[:], t16[:])
                dr = gp.tile([1, 64], f32, tag=f"dummy{dirtag}")
                nc.sync.dma_start(dr[:], agg[0:1, 0:64])
                nc.gpsimd.dma_scatter_add(
                    agg[:], t32[:],
                    idx_tiles[skey][:, k * COLS:(k + 1) * COLS],
                    CALL, CALL, D, queue_num=1 if dirtag == "i" else 2)

            # interleave the 8 (direction, chunk) streams round-robin so that
            # consecutive same-buffer scatters sharing a dst row are far
            # apart in time (adjacent same-stream calls are dst-disjoint
            # splits of one rank group or consecutive ranks)
            ch_base = (0, caps[0], caps[0] + caps[1], caps[0] + caps[1] + caps[2])
            for r in range(max(caps)):
                for c in range(NCHUNK):
                    if r < caps[c]:
                        emit_call(ch_base[c] + r, c, "gi", "si", agg_in, "i")
                        emit_call(ch_base[c] + r, c, "go", "so", agg_out, "o")

            # epilogue per 128-node tile
            for t in range(98):
                ai = ep.tile([128, D], f32, tag="ai")
                ao = ep.tile([128, D], f32, tag="ao")
                nc.sync.dma_start(ai[:], agg_in[t * 128:(t + 1) * 128, :])
                nc.sync.dma_start(ao[:], agg_out[t * 128:(t + 1) * 128, :])
                # scale by inv degree (per-partition scalar)
                nc.vector.tensor_scalar(ai[:], ai[:], ivi_s[:, t:t + 1], None,
                                        mybir.AluOpType.mult)
                nc.vector.tensor_scalar(ao[:], ao[:], ivo_s[:, t:t + 1], None,
                                        mybir.AluOpType.mult)
                # own-shard x tile: load f16, convert to f32
                xt16 = ep.tile([128, D], f16, tag="xt16")
                nc.sync.dma_start(xt16[:], xs[t * 128:(t + 1) * 128, :])
                xt = ep.tile([128, D], f32, tag="xt")
                nc.vector.tensor_copy(xt[:], xt16[:])
                # transpose all three activations
                pt = pp.tile([128, D], f32, tag="pt")
                nc.tensor.matmul(pt[:], ai[:], id_s[:], start=True, stop=True,
                                 is_transpose=True)
                aiT = ep.tile([128, D], f32, tag="aiT")
                nc.vector.tensor_copy(aiT[:], pt[:])
                pt2 = pp.tile([128, D], f32, tag="pt")
                nc.tensor.matmul(pt2[:], ao[:], id_s[:], start=True, stop=True,
                                 is_transpose=True)
                aoT = ep.tile([128, D], f32, tag="aoT")
                nc.vector.tensor_copy(aoT[:], pt2[:])
                pt3 = pp.tile([128, D], f32, tag="pt")
                nc.tensor.matmul(pt3[:], xt[:], id_s[:], start=True, stop=True,
                                 is_transpose=True)
                xtT = ep.tile([128, D], f32, tag="xtT")
                nc.vector.tensor_copy(xtT[:], pt3[:])
                # y = W_self.T @ xT + W1.T @ aiT + W2.T @ aoT   [feat_out, nodes]
                y = pp.tile([128, 128], f32, tag="y")
                nc.tensor.matmul(y[:], Ws_s[:], xtT[:], start=True, stop=False)
                nc.tensor.matmul(y[:], W1_s[:], aiT[:], start=False, stop=False)
                nc.tensor.matmul(y[:], W2_s[:], aoT[:], start=False, stop=True)
                ysb = ep.tile([128, 128], f32, tag="ysb")
                nc.vector.tensor_scalar(ysb[:], y[:], b_s[:, 0:1], None,
                                        mybir.AluOpType.add)
                # transpose back to [nodes, feat], convert to f16
                po = pp.tile([128, 128], f32, tag="po")
                nc.tensor.matmul(po[:], ysb[:], id_s[:], start=True, stop=True,
                                 is_transpose=True)
                osb = ep.tile([128, 128], f16, tag="osb")
                nc.vector.tensor_copy(osb[:], po[:])
                nc.sync.dma_start(out[t * 128:(t + 1) * 128, :], osb[:])

    nc.compile()
    return nc


# --------------------------------------------------------------- AOT setup --

_AOT = {}


def _aot_compile(caps):
    t0 = _time.perf_counter()
    import jax
    from jax.sharding import Mesh, PartitionSpec, NamedSharding
    from jax.experimental.shard_map import shard_map
    from concourse import bass2jax, mybir

    nc = _build_program(caps)
    t0 = _log("aot: build+bass-compile", t0)

    bass2jax.install_neuronx_cc_hook()
    partition_name = nc.partition_id_tensor.name if nc.partition_id_tensor else None
    in_names, out_names, out_avals, zero_outs = [], [], [], []
    for alloc in nc.m.functions[0].allocations:
        if not isinstance(alloc, mybir.MemoryLocationSet):
            continue
        name = alloc.memorylocations[0].name
        if alloc.kind == "ExternalInput":
            if name != partition_name:
                in_names.append(name)
        elif alloc.kind == "ExternalOutput":
            shape = tuple(alloc.tensor_shape)
            dtype = mybir.dt.np(alloc.dtype)
            out_names.append(name)
            out_avals.append(jax.core.ShapedArray(shape, dtype))
            zero_outs.append(np.zeros(shape, dtype))
    n_params = len(in_names)
    n_outs = len(out_avals)
    in_names_full = in_names + out_names + ([partition_name] if partition_name else [])

    def _body(*args):
        operands = list(args)
        if partition_name is not None:
            operands.append(bass2jax.partition_id_tensor())
        outs = bass2jax._bass_exec_p.bind(
            *operands,
            out_avals=tuple(out_avals),
            in_names=tuple(in_names_full),
            out_names=tuple(out_names),
            lowering_input_output_aliases=(),
            sim_require_finite=True,
            sim_require_nnan=True,
            nc=nc,
        )
        return tuple(outs)

    devices = jax.devices()[:P]
    mesh = Mesh(np.asarray(devices), ("core",))
    in_specs = (PartitionSpec("core"),) * (n_params + n_outs)
    out_specs = (PartitionSpec("core"),) * n_outs
    donate = tuple(range(n_params, n_params + n_outs))
    sharded = jax.jit(
        shard_map(_body, mesh=mesh, in_specs=in_specs, out_specs=out_specs,
                  check_rep=False),
        donate_argnums=donate, keep_unused=True)

    # abstract shapes: per-core input shapes concatenated over cores on axis 0
    shape_of = {}
    for alloc in nc.m.functions[0].allocations:
        if isinstance(alloc, mybir.MemoryLocationSet) and alloc.kind == "ExternalInput":
            shape_of[alloc.memorylocations[0].name] = (
                tuple(alloc.tensor_shape), mybir.dt.np(alloc.dtype))
    abstract = []
    for name in in_names:
        shp, dt = shape_of[name]
        abstract.append(jax.ShapeDtypeStruct((P * shp[0],) + shp[1:], dt))
    for z in zero_outs:
        abstract.append(jax.ShapeDtypeStruct((P * z.shape[0],) + z.shape[1:], z.dtype))
    compiled = sharded.lower(*abstract).compile()
    t0 = _log("aot: neff-compile", t0)

    sharding = NamedSharding(mesh, PartitionSpec("core"))
    _AOT.update(dict(
        caps=tuple(caps), nc=nc, compiled=compiled, in_names=in_names,
        out_names=out_names, zero_outs=zero_outs, sharding=sharding,
        dz=None, jax=jax))
    _make_zeros()
    _log("aot: zero-put", t0)


def _make_zeros():
    jax = _AOT["jax"]
    _AOT["dz"] = [
        jax.device_put(
            np.zeros((P * z.shape[0],) + z.shape[1:], z.dtype), _AOT["sharding"])
        for z in _AOT["zero_outs"]]
    jax.block_until_ready(_AOT["dz"])


import os
if not os.environ.get("KERNEL_NO_AOT"):
    try:
        _aot_compile(CAPS)
    except Exception as _e:  # pragma: no cover - fall through to lazy compile
        print(f"[kernel] AOT compile failed ({_e!r}); will compile lazily",
              file=sys.stderr)
        _AOT.clear()


# ------------------------------------------------------------------ kernel --

def kernel(x, W_self, b_self, W_s2d, b_s2d, W_d2s, b_d2s, edge_index):
    t0 = _time.perf_counter()
    x = np.ascontiguousarray(x, np.float32)
    W_self = np.asarray(W_self, np.float32)
    b_self = np.asarray(b_self, np.float32)
    W_s2d = np.asarray(W_s2d, np.float32)
    b_s2d = np.asarray(b_s2d, np.float32)
    W_d2s = np.asarray(W_d2s, np.float32)
    b_d2s = np.asarray(b_d2s, np.float32)
    src = np.asarray(edge_index[0], np.int64)
    dst = np.asarray(edge_index[1], np.int64)

    jax = _AOT.get("jax")
    if jax is None:
        import jax  # lazy path
    sharding = _AOT.get("sharding")

    # start x upload first (overlaps with planning)
    x16 = np.zeros((NFULL, D), np.float16)
    x16v = x16.reshape(P, NSH_PAD, D)
    x16v[:, :NSH] = x.reshape(P, NSH, D).astype(np.float16)
    dev_x = jax.device_put(x16, sharding) if sharding is not None else None
    t0 = _log("x-convert+put", t0)

    # plans (vectorized)
    caps = _AOT.get("caps", CAPS)
    gi, si, need_i = _plan_direction(src, dst, caps)
    go, so, need_o = _plan_direction(dst, src, caps)
    if gi is None or go is None:
        need = np.maximum(need_i, need_o).max(axis=0)
        caps = tuple(int(v) for v in np.maximum(np.asarray(caps), need + 2))
        print(f"[kernel] capacity exceeded; recompiling with caps={caps}",
              file=sys.stderr)
        _aot_compile(caps)
        sharding = _AOT["sharding"]
        dev_x = jax.device_put(x16, sharding)
        gi, si, need_i = _plan_direction(src, dst, caps)
        go, so, need_o = _plan_direction(dst, src, caps)
    t0 = _log("plan", t0)

    deg_in = np.bincount(dst, minlength=N).astype(np.float32)
    deg_out = np.bincount(src, minlength=N).astype(np.float32)
    inv_in = 1.0 / np.maximum(deg_in, 1.0)
    inv_out = 1.0 / np.maximum(deg_out, 1.0)
    W1 = (1.0 - ALPHA) * W_s2d
    W2 = ALPHA * W_d2s
    b_tot = b_self + (1.0 - ALPHA) * b_s2d + ALPHA * b_d2s

    # per-core concatenated inputs, in in_names order
    nc_dir = int(sum(caps))
    invs = {"inv_in": inv_in, "inv_out": inv_out}
    packed_inv = {}
    for k, v in invs.items():
        a = np.zeros((P, NSH_PAD), np.float32)
        a[:, :NSH] = v.reshape(P, NSH)
        packed_inv[k] = np.ascontiguousarray(
            a.reshape(P, 98, 128).transpose(0, 2, 1)).reshape(P * 128, 98)
    # wrap per core: idx i -> [i % 16, i // 16], then stack cores on axis 0
    idx_cat = {}
    for k, v in (("gidx_in", gi), ("sidx_in", si),
                 ("gidx_out", go), ("sidx_out", so)):
        idx_cat[k] = np.ascontiguousarray(
            v.reshape(P, nc_dir * COLS, 16).transpose(0, 2, 1)
        ).reshape(P * 16, nc_dir * COLS)

    rep = {
        "W_self": np.broadcast_to(W_self, (P,) + W_self.shape).reshape(P * D, D),
        "W1": np.broadcast_to(W1, (P,) + W1.shape).reshape(P * D, D),
        "W2": np.broadcast_to(W2, (P,) + W2.shape).reshape(P * D, D),
        "b": np.broadcast_to(b_tot.reshape(D, 1), (P, D, 1)).reshape(P * D, 1),
        "ident": np.tile(np.eye(D, dtype=np.float32), (P, 1)),
    }
    t0 = _log("pack-inputs", t0)

    if not _AOT:
        _aot_compile(caps)
        dev_x = _AOT["jax"].device_put(x16, _AOT["sharding"])
    jax = _AOT["jax"]
    sharding = _AOT["sharding"]
    compiled = _AOT["compiled"]

    dev_args = []
    for name in _AOT["in_names"]:
        if name == "x_sh":
            dev_args.append(dev_x if dev_x is not None
                            else jax.device_put(x16, sharding))
        elif name in idx_cat:
            dev_args.append(jax.device_put(
                np.ascontiguousarray(idx_cat[name]), sharding))
        elif name in packed_inv:
            dev_args.append(jax.device_put(packed_inv[name], sharding))
        else:
            dev_args.append(jax.device_put(rep[name], sharding))
    if _AOT["dz"] is None:
        _make_zeros()
    dz = _AOT["dz"]
    _AOT["dz"] = None  # consumed by donation
    jax.block_until_ready(dev_args)
    t0 = _log("upload", t0)

    outs = compiled(*dev_args, *dz)
    jax.block_until_ready(outs)
    t0 = _log("exec", t0)

    out16 = np.asarray(outs[0]).reshape(P, NSH_PAD, D)
    res = np.empty((N, D), np.float32)
    res_v = res.reshape(P, NSH, D)
    res_v[:] = out16[:, :NSH].astype(np.float32)
    _log("fetch+convert", t0)
    return res


# revision 21
# speedup vs baseline: 18.6457x; 1.1608x over previous
import sys
sys.path.insert(0, "/opt/trn_rl_repo")
import time as _time
import numpy as np

N = 100000
E = 800000
D = 128
P = 8
NSH = 12500          # nodes per core
NSH_PAD = 12544      # 98 * 128
NFULL = P * NSH_PAD  # 100352 rows in allgathered x
ALPHA = 0.5
CALL = 896           # idxs per call (56 idx cols per call: ring-safe)
COLS = CALL // 16
# gather chunks over x_full rows (int16 idx limit 32767): chunk c = rows
# [32768c, 32768c+32768); chunk 3 is short (2048 rows)
NCHUNK = 4
CHUNK_ROWS = (32768, 32768, 32768, NFULL - 3 * 32768)
# per-direction per-chunk call capacity (fixed program shape; ~20% margin
# over the expected rank-grouped call count for E/P random edges)
CAPS = (50, 50, 50, 8)
NC_DIR = sum(CAPS)          # calls per direction
CHUNK_BASE = (0, CAPS[0], CAPS[0] + CAPS[1], CAPS[0] + CAPS[1] + CAPS[2])

_verbose = True


def _log(tag, t0):
    if _verbose:
        now = _time.perf_counter()
        print(f"[kernel-timing] {tag}: {now - t0:.3f}s", file=sys.stderr)
    return _time.perf_counter()


# ---------------------------------------------------------------- planner ---

def _plan_direction(gat, seg, caps):
    """Vectorized duplicate-free call plan for one direction.

    gat: global gather node per edge; seg: global segment (scatter) node.
    Returns (gflat, sflat) int16 arrays [P, NC_DIR*CALL] filled with -1 pads,
    plus per-(core,chunk) needed call counts [P, NCHUNK].
    Within one call every scatter target is unique (edges grouped by
    round-robin rank within their (chunk, segment)), so no same-row RMW
    races inside a call.
    """
    seg = seg.astype(np.int32)
    gat = gat.astype(np.int32)
    core = seg // NSH
    loc = seg - core * NSH
    q = gat // NSH
    row = q * NSH_PAD + (gat - q * NSH)
    chunk = row >> 15
    lidx = row & 32767

    cc = core * NCHUNK + chunk                 # 0..31
    # sort by (core, chunk, seg); rank = run position within equal seg
    k1 = cc * (1 << 17) + seg                  # < 2^22, int32 radix sort
    o1 = np.argsort(k1, kind="stable")
    k1s = k1[o1]
    first = np.empty(E, bool)
    first[0] = True
    np.not_equal(k1s[1:], k1s[:-1], out=first[1:])
    ar = np.arange(E, dtype=np.int64)
    idx_first = np.maximum.accumulate(np.where(first, ar, 0))
    rank = (ar - idx_first).astype(np.int32)
    # sort by (core, chunk, rank) stable -> final edge order
    k2 = cc[o1] * E + rank                     # < 2^25, int32 radix sort
    o2 = np.argsort(k2, kind="stable")
    k2s = k2[o2]
    of = o1[o2]
    # position within each (core, chunk, rank) run
    first2 = np.empty(E, bool)
    first2[0] = True
    np.not_equal(k2s[1:], k2s[:-1], out=first2[1:])
    idx_first2 = np.maximum.accumulate(np.where(first2, ar, 0))
    posr = ar - idx_first2
    call_in_run = posr // CALL
    slot = posr - call_in_run * CALL
    # per-run call counts -> per-(core,chunk) cumulative call base per rank
    run_starts = np.flatnonzero(first2)
    run_lens = np.diff(np.r_[run_starts, E])
    run_calls = (run_lens + CALL - 1) // CALL
    run_cc = cc[of[run_starts]]
    # cumulative calls of earlier ranks within same (core,chunk)
    csum = np.cumsum(run_calls) - run_calls
    cc_first_run = np.empty(run_cc.size, bool)
    cc_first_run[0] = True
    np.not_equal(run_cc[1:], run_cc[:-1], out=cc_first_run[1:])
    arr_r = np.arange(run_cc.size)
    idx_first_cc = np.maximum.accumulate(np.where(cc_first_run, arr_r, 0))
    run_base = csum - csum[idx_first_cc]
    needed = np.zeros((P, NCHUNK), np.int64)
    last_of_cc = np.r_[cc_first_run[1:], True]
    needed[run_cc[last_of_cc] // NCHUNK, run_cc[last_of_cc] % NCHUNK] = \
        (run_base + run_calls)[last_of_cc]
    if np.any(needed > np.asarray(caps)[None, :]):
        return None, None, needed
    # flat destination index
    call_idx = run_base[np.cumsum(first2) - 1] + call_in_run
    ch_base = np.asarray(
        [0, caps[0], caps[0] + caps[1], caps[0] + caps[1] + caps[2]])
    nc_dir = int(sum(caps))
    core_f = core[of]
    chunk_f = chunk[of]
    flat = ((core_f * nc_dir + ch_base[chunk_f] + call_idx) * CALL + slot)
    # pads gather row 0 of their chunk and scatter into agg row NSH (a
    # discarded pad row); same-row pad adds may race but are never read
    gflat = np.zeros(P * nc_dir * CALL, np.int16)
    sflat = np.full(P * nc_dir * CALL, NSH, np.int16)
    gflat[flat] = lidx[of].astype(np.int16)
    sflat[flat] = loc[of].astype(np.int16)
    return (gflat.reshape(P, nc_dir * CALL),
            sflat.reshape(P, nc_dir * CALL), needed)


def _wrap16(a):
    # idx i -> [i % 16, i // 16]
    return np.ascontiguousarray(a.reshape(-1, 16).T)


# ------------------------------------------------------------- device prog --

def _build_program(caps):
    from concourse import bacc, tile, mybir, library_config

    f32 = mybir.dt.float32
    f16 = mybir.dt.float16
    i16 = mybir.dt.int16
    nc = bacc.Bacc("TRN2", target_bir_lowering=False, debug=False,
                   num_swdge_queues=3, num_devices=P)

    nc_dir = int(sum(caps))
    xs = nc.dram_tensor("x_sh", [NSH_PAD, D], f16, kind="ExternalInput")
    gii = nc.dram_tensor("gidx_in", [16, nc_dir * COLS], i16, kind="ExternalInput")
    sii = nc.dram_tensor("sidx_in", [16, nc_dir * COLS], i16, kind="ExternalInput")
    gio = nc.dram_tensor("gidx_out", [16, nc_dir * COLS], i16, kind="ExternalInput")
    sio = nc.dram_tensor("sidx_out", [16, nc_dir * COLS], i16, kind="ExternalInput")
    ivi = nc.dram_tensor("inv_in", [128, 98], f32, kind="ExternalInput")
    ivo = nc.dram_tensor("inv_out", [128, 98], f32, kind="ExternalInput")
    Ws = nc.dram_tensor("W_self", [D, D], f32, kind="ExternalInput")
    W1 = nc.dram_tensor("W1", [D, D], f32, kind="ExternalInput")
    W2 = nc.dram_tensor("W2", [D, D], f32, kind="ExternalInput")
    bt = nc.dram_tensor("b", [D, 1], f32, kind="ExternalInput")
    idn = nc.dram_tensor("ident", [D, D], f32, kind="ExternalInput")
    x_bounce = nc.dram_tensor("x_bounce", [NSH_PAD, D], f16)
    # NOTE: addr_space="Shared" for the AllGather output desyncs the mesh
    # under the axon PJRT path; plain DRAM works (slower CC but tiny anyway).
    x_full = nc.dram_tensor("x_full", [NFULL, D], f16)
    agg_in = nc.dram_tensor("agg_in", [NSH_PAD, D], f32)
    agg_out = nc.dram_tensor("agg_out", [NSH_PAD, D], f32)
    out = nc.dram_tensor("out", [NSH_PAD, D], f16, kind="ExternalOutput")

    with tile.TileContext(nc) as tc:
        nc.gpsimd.load_library(library_config.mlp)
        with tc.tile_pool(name="const", bufs=1) as cp, \
             tc.tile_pool(name="gt", bufs=2) as gp, \
             tc.tile_pool(name="ep", bufs=3) as ep, \
             tc.tile_pool(name="ps", bufs=2, space="PSUM") as pp:
            # shard -> bounce -> allgather into x_full
            nc.sync.dma_start(x_bounce[:], xs[:])
            nc.gpsimd.collective_compute(
                "AllGather", mybir.AluOpType.bypass,
                replica_groups=[list(range(P))],
                ins=[x_bounce.ap().opt()],
                outs=[x_full.ap().opt()],
            )

            # index tiles: load 16 rows, replicate to 128 partitions on device
            idx_tiles = {}
            for nm, src in (("gi", gii), ("si", sii), ("go", gio), ("so", sio)):
                t = cp.tile([128, nc_dir * COLS], i16, tag=f"idx_{nm}")
                nc.sync.dma_start(t[0:16, :], src[:])
                nc.sync.dma_start(t[16:32, :], src[:])
                nc.sync.dma_start(t[32:64, :], t[0:32, :])
                nc.sync.dma_start(t[64:128, :], t[0:64, :])
                idx_tiles[nm] = t
            ivi_s = cp.tile([128, 98], f32)
            ivo_s = cp.tile([128, 98], f32)
            nc.sync.dma_start(ivi_s[:], ivi[:])
            nc.sync.dma_start(ivo_s[:], ivo[:])
            Ws_s = cp.tile([D, D], f32)
            W1_s = cp.tile([D, D], f32)
            W2_s = cp.tile([D, D], f32)
            b_s = cp.tile([D, 1], f32)
            id_s = cp.tile([D, D], f32)
            nc.sync.dma_start(Ws_s[:], Ws[:])
            nc.sync.dma_start(W1_s[:], W1[:])
            nc.sync.dma_start(W2_s[:], W2[:])
            nc.sync.dma_start(b_s[:], bt[:])
            nc.sync.dma_start(id_s[:], idn[:])

            # zero agg buffers from an SBUF zero tile
            zt = cp.tile([128, NSH_PAD], f32)
            nc.vector.memset(zt[:], 0.0)
            for t in range(98):
                nc.sync.dma_start(agg_in[t * 128:(t + 1) * 128, :],
                                  zt[:, t * 128:(t + 1) * 128])
                nc.sync.dma_start(agg_out[t * 128:(t + 1) * 128, :],
                                  zt[:, t * 128:(t + 1) * 128])

            # gather + convert + scatter chains; a dummy reader of the agg
            # buffer between consecutive same-buffer scatters forces each
            # scatter's DMA to complete before the next starts (cross-call
            # same-row RMWs on different DMA engines would otherwise race).
            def emit_call(k, c, gkey, skey, agg, dirtag):
                t16 = gp.tile([128, CALL // 128, D], f16, tag="g16")
                if c < 3:
                    src = x_full[c * 32768:(c + 1) * 32768, :]
                else:
                    src = x_full[3 * 32768:NFULL, :]
                nc.gpsimd.dma_gather(
                    t16[:], src,
                    idx_tiles[gkey][:, k * COLS:(k + 1) * COLS],
                    CALL, CALL, D, queue_num=0)
                t32 = gp.tile([128, CALL // 128, D], f32, tag="g32")
                nc.vector.tensor_copy(t32[:], t16[:])
                dr = gp.tile([1, 64], f32, tag=f"dummy{dirtag}")
                nc.sync.dma_start(dr[:], agg[0:1, 0:64])
                nc.gpsimd.dma_scatter_add(
                    agg[:], t32[:],
                    idx_tiles[skey][:, k * COLS:(k + 1) * COLS],
                    CALL, CALL, D, queue_num=1 if dirtag == "i" else 2)

            # interleave the 8 (direction, chunk) streams round-robin so that
            # consecutive same-buffer scatters sharing a dst row are far
            # apart in time (adjacent same-stream calls are dst-disjoint
            # splits of one rank group or consecutive ranks)
            ch_base = (0, caps[0], caps[0] + caps[1], caps[0] + caps[1] + caps[2])
            for r in range(max(caps)):
                for c in range(NCHUNK):
                    if r < caps[c]:
                        emit_call(ch_base[c] + r, c, "gi", "si", agg_in, "i")
                        emit_call(ch_base[c] + r, c, "go", "so", agg_out, "o")

            # epilogue per 128-node tile
            for t in range(98):
                ai = ep.tile([128, D], f32, tag="ai")
                ao = ep.tile([128, D], f32, tag="ao")
                nc.sync.dma_start(ai[:], agg_in[t * 128:(t + 1) * 128, :])
                nc.sync.dma_start(ao[:], agg_out[t * 128:(t + 1) * 128, :])
                # scale by inv degree (per-partition scalar)
                nc.vector.tensor_scalar(ai[:], ai[:], ivi_s[:, t:t + 1], None,
                                        mybir.AluOpType.mult)
                nc.vector.tensor_scalar(ao[:], ao[:], ivo_s[:, t:t + 1], None,
                                        mybir.AluOpType.mult)
                # own-shard x tile: load f16, convert to f32
                xt16 = ep.tile([128, D], f16, tag="xt16")
                nc.sync.dma_start(xt16[:], xs[t * 128:(t + 1) * 128, :])
                xt = ep.tile([128, D], f32, tag="xt")
                nc.vector.tensor_copy(xt[:], xt16[:])
                # transpose all three activations
                pt = pp.tile([128, D], f32, tag="pt")
                nc.tensor.matmul(pt[:], ai[:], id_s[:], start=True, stop=True,
                                 is_transpose=True)
                aiT = ep.tile([128, D], f32, tag="aiT")
                nc.vector.tensor_copy(aiT[:], pt[:])
                pt2 = pp.tile([128, D], f32, tag="pt")
                nc.tensor.matmul(pt2[:], ao[:], id_s[:], start=True, stop=True,
                                 is_transpose=True)
                aoT = ep.tile([128, D], f32, tag="aoT")
                nc.vector.tensor_copy(aoT[:], pt2[:])
                pt3 = pp.tile([128, D], f32, tag="pt")
                nc.tensor.matmul(pt3[:], xt[:], id_s[:], start=True, stop=True,
                                 is_transpose=True)
                xtT = ep.tile([128, D], f32, tag="xtT")
                nc.vector.tensor_copy(xtT[:], pt3[:])
                # y = W_self.T @ xT + W1.T @ aiT + W2.T @ aoT   [feat_out, nodes]
                y = pp.tile([128, 128], f32, tag="y")
                nc.tensor.matmul(y[:], Ws_s[:], xtT[:], start=True, stop=False)
                nc.tensor.matmul(y[:], W1_s[:], aiT[:], start=False, stop=False)
                nc.tensor.matmul(y[:], W2_s[:], aoT[:], start=False, stop=True)
                ysb = ep.tile([128, 128], f32, tag="ysb")
                nc.vector.tensor_scalar(ysb[:], y[:], b_s[:, 0:1], None,
                                        mybir.AluOpType.add)
                # transpose back to [nodes, feat], convert to f16
                po = pp.tile([128, 128], f32, tag="po")
                nc.tensor.matmul(po[:], ysb[:], id_s[:], start=True, stop=True,
                                 is_transpose=True)
                osb = ep.tile([128, 128], f16, tag="osb")
                nc.vector.tensor_copy(osb[:], po[:])
                nc.sync.dma_start(out[t * 128:(t + 1) * 128, :], osb[:])

    nc.compile()
    return nc


# --------------------------------------------------------------- AOT setup --

_AOT = {}


def _aot_compile(caps):
    t0 = _time.perf_counter()
    import jax
    from jax.sharding import Mesh, PartitionSpec, NamedSharding
    from jax.experimental.shard_map import shard_map
    from concourse import bass2jax, mybir

    nc = _build_program(caps)
    t0 = _log("aot: build+bass-compile", t0)

    bass2jax.install_neuronx_cc_hook()
    partition_name = nc.partition_id_tensor.name if nc.partition_id_tensor else None
    in_names, out_names, out_avals, zero_outs = [], [], [], []
    for alloc in nc.m.functions[0].allocations:
        if not isinstance(alloc, mybir.MemoryLocationSet):
            continue
        name = alloc.memorylocations[0].name
        if alloc.kind == "ExternalInput":
            if name != partition_name:
                in_names.append(name)
        elif alloc.kind == "ExternalOutput":
            shape = tuple(alloc.tensor_shape)
            dtype = mybir.dt.np(alloc.dtype)
            out_names.append(name)
            out_avals.append(jax.core.ShapedArray(shape, dtype))
            zero_outs.append(np.zeros(shape, dtype))
    n_params = len(in_names)
    n_outs = len(out_avals)
    in_names_full = in_names + out_names + ([partition_name] if partition_name else [])

    def _body(*args):
        operands = list(args)
        if partition_name is not None:
            operands.append(bass2jax.partition_id_tensor())
        outs = bass2jax._bass_exec_p.bind(
            *operands,
            out_avals=tuple(out_avals),
            in_names=tuple(in_names_full),
            out_names=tuple(out_names),
            lowering_input_output_aliases=(),
            sim_require_finite=True,
            sim_require_nnan=True,
            nc=nc,
        )
        return tuple(outs)

    devices = jax.devices()[:P]
    mesh = Mesh(np.asarray(devices), ("core",))
    in_specs = (PartitionSpec("core"),) * (n_params + n_outs)
    out_specs = (PartitionSpec("core"),) * n_outs
    donate = tuple(range(n_params, n_params + n_outs))
    sharded = jax.jit(
        shard_map(_body, mesh=mesh, in_specs=in_specs, out_specs=out_specs,
                  check_rep=False),
        donate_argnums=donate, keep_unused=True)

    # abstract shapes: per-core input shapes concatenated over cores on axis 0
    shape_of = {}
    for alloc in nc.m.functions[0].allocations:
        if isinstance(alloc, mybir.MemoryLocationSet) and alloc.kind == "ExternalInput":
            shape_of[alloc.memorylocations[0].name] = (
                tuple(alloc.tensor_shape), mybir.dt.np(alloc.dtype))
    abstract = []
    for name in in_names:
        shp, dt = shape_of[name]
        abstract.append(jax.ShapeDtypeStruct((P * shp[0],) + shp[1:], dt))
    for z in zero_outs:
        abstract.append(jax.ShapeDtypeStruct((P * z.shape[0],) + z.shape[1:], z.dtype))
    compiled = sharded.lower(*abstract).compile()
    t0 = _log("aot: neff-compile", t0)

    sharding = NamedSharding(mesh, PartitionSpec("core"))
    _AOT.update(dict(
        caps=tuple(caps), nc=nc, compiled=compiled, in_names=in_names,
        out_names=out_names, zero_outs=zero_outs, sharding=sharding,
        dz=None, jax=jax))
    _make_zeros()
    _log("aot: zero-put", t0)


def _make_zeros():
    jax = _AOT["jax"]
    _AOT["dz"] = [
        jax.device_put(
            np.zeros((P * z.shape[0],) + z.shape[1:], z.dtype), _AOT["sharding"])
        for z in _AOT["zero_outs"]]
    jax.block_until_ready(_AOT["dz"])


import os
if not os.environ.get("KERNEL_NO_AOT"):
    try:
        _aot_compile(CAPS)
    except Exception as _e:  # pragma: no cover - fall through to lazy compile
        print(f"[kernel] AOT compile failed ({_e!r}); will compile lazily",
              file=sys.stderr)
        _AOT.clear()


# ------------------------------------------------------------------ kernel --

def kernel(x, W_self, b_self, W_s2d, b_s2d, W_d2s, b_d2s, edge_index):
    t0 = _time.perf_counter()
    x = np.ascontiguousarray(x, np.float32)
    W_self = np.asarray(W_self, np.float32)
    b_self = np.asarray(b_self, np.float32)
    W_s2d = np.asarray(W_s2d, np.float32)
    b_s2d = np.asarray(b_s2d, np.float32)
    W_d2s = np.asarray(W_d2s, np.float32)
    b_d2s = np.asarray(b_d2s, np.float32)
    src = np.asarray(edge_index[0], np.int64)
    dst = np.asarray(edge_index[1], np.int64)

    jax = _AOT.get("jax")
    if jax is None:
        import jax  # lazy path
    sharding = _AOT.get("sharding")

    # plans (vectorized, one thread per direction; each thread starts its
    # index uploads as soon as they are packed); x conversion + upload runs
    # on the main thread meanwhile
    from concurrent.futures import ThreadPoolExecutor
    caps = _AOT.get("caps", CAPS)
    nc_dir = int(sum(caps))
    staged = {}

    def _pack_idx(v):
        return np.ascontiguousarray(
            v.reshape(P, nc_dir * COLS, 16).transpose(0, 2, 1)
        ).reshape(P * 16, nc_dir * COLS)

    def _do_dir(tag, gat, seg):
        g, s, need = _plan_direction(gat, seg, caps)
        if g is None:
            return need
        for key, arr in ((f"gidx_{tag}", g), (f"sidx_{tag}", s)):
            packed = _pack_idx(arr)
            staged[key] = (jax.device_put(packed, sharding)
                           if sharding is not None else packed)
        return None

    with ThreadPoolExecutor(2) as ex:
        f_in = ex.submit(_do_dir, "in", src, dst)
        f_out = ex.submit(_do_dir, "out", dst, src)
        # main thread: x conversion + upload, degrees, weights, inv packing
        x16 = np.zeros((NFULL, D), np.float16)
        x16v = x16.reshape(P, NSH_PAD, D)
        x16v[:, :NSH] = x.reshape(P, NSH, D).astype(np.float16)
        dev_x = jax.device_put(x16, sharding) if sharding is not None else None
        deg_in = np.bincount(dst, minlength=N).astype(np.float32)
        deg_out = np.bincount(src, minlength=N).astype(np.float32)
        inv_in = 1.0 / np.maximum(deg_in, 1.0)
        inv_out = 1.0 / np.maximum(deg_out, 1.0)
        W1 = (1.0 - ALPHA) * W_s2d
        W2 = ALPHA * W_d2s
        b_tot = b_self + (1.0 - ALPHA) * b_s2d + ALPHA * b_d2s
        for k, v in (("inv_in", inv_in), ("inv_out", inv_out)):
            a = np.zeros((P, NSH_PAD), np.float32)
            a[:, :NSH] = v.reshape(P, NSH)
            packed = np.ascontiguousarray(
                a.reshape(P, 98, 128).transpose(0, 2, 1)).reshape(P * 128, 98)
            staged[k] = (jax.device_put(packed, sharding)
                         if sharding is not None else packed)
        rep = {
            "W_self": np.broadcast_to(W_self, (P,) + W_self.shape).reshape(P * D, D),
            "W1": np.broadcast_to(W1, (P,) + W1.shape).reshape(P * D, D),
            "W2": np.broadcast_to(W2, (P,) + W2.shape).reshape(P * D, D),
            "b": np.broadcast_to(b_tot.reshape(D, 1), (P, D, 1)).reshape(P * D, 1),
            "ident": np.tile(np.eye(D, dtype=np.float32), (P, 1)),
        }
        for k, v in rep.items():
            staged[k] = (jax.device_put(np.ascontiguousarray(v), sharding)
                         if sharding is not None else v)
        need_i = f_in.result()
        need_o = f_out.result()

    if need_i is not None or need_o is not None:
        # capacity exceeded: recompile with room and redo plans serially
        zero = np.zeros((P, NCHUNK), np.int64)
        need = np.maximum(need_i if need_i is not None else zero,
                          need_o if need_o is not None else zero).max(axis=0)
        caps = tuple(int(v) for v in np.maximum(np.asarray(caps), need + 2))
        print(f"[kernel] capacity exceeded; recompiling with caps={caps}",
              file=sys.stderr)
        _aot_compile(caps)
        sharding = _AOT["sharding"]
        jax = _AOT["jax"]
        nc_dir = int(sum(caps))
        dev_x = jax.device_put(x16, sharding)
        gi, si, _ = _plan_direction(src, dst, caps)
        go, so, _ = _plan_direction(dst, src, caps)
        for key, arr in (("gidx_in", gi), ("sidx_in", si),
                         ("gidx_out", go), ("sidx_out", so)):
            staged[key] = jax.device_put(_pack_idx(arr), sharding)
        for k, v in (("inv_in", inv_in), ("inv_out", inv_out)):
            a = np.zeros((P, NSH_PAD), np.float32)
            a[:, :NSH] = v.reshape(P, NSH)
            staged[k] = jax.device_put(np.ascontiguousarray(
                a.reshape(P, 98, 128).transpose(0, 2, 1)).reshape(P * 128, 98),
                sharding)
        for k, v in rep.items():
            staged[k] = jax.device_put(np.ascontiguousarray(v), sharding)
    t0 = _log("plan+pack+put", t0)

    if not _AOT:
        _aot_compile(caps)
        jax = _AOT["jax"]
        sharding = _AOT["sharding"]
        dev_x = jax.device_put(x16, sharding)
        for k, v in list(staged.items()):
            staged[k] = jax.device_put(v, sharding)
    compiled = _AOT["compiled"]

    dev_args = [dev_x if name == "x_sh" else staged[name]
                for name in _AOT["in_names"]]
    if _AOT["dz"] is None:
        _make_zeros()
    dz = _AOT["dz"]
    _AOT["dz"] = None  # consumed by donation
    jax.block_until_ready(dev_args)
    t0 = _log("upload", t0)

    try:
        outs = compiled(*dev_args, *dz)
        jax.block_until_ready(outs)
    except Exception as e:  # transient device failure: one retry
        print(f"[kernel] exec failed ({e!r}); retrying once", file=sys.stderr)
        _make_zeros()
        dz = _AOT["dz"]
        _AOT["dz"] = None
        outs = compiled(*dev_args, *dz)
        jax.block_until_ready(outs)
    t0 = _log("exec", t0)

    # fetch the 8 output shards in parallel and convert per shard
    res = np.empty((N, D), np.float32)
    res_v = res.reshape(P, NSH, D)
    shards = list(outs[0].addressable_shards)

    def _fetch(sh):
        c = sh.index[0].start // NSH_PAD if sh.index[0].start else 0
        res_v[c] = np.asarray(sh.data)[:NSH].astype(np.float32)

    with ThreadPoolExecutor(P) as ex:
        list(ex.map(_fetch, shards))
    _log("fetch+convert", t0)
    return res


# revision 29
# speedup vs baseline: 20.8528x; 1.1184x over previous
import sys
sys.path.insert(0, "/opt/trn_rl_repo")
import time as _time
import numpy as np

N = 100000
E = 800000
D = 128
P = 8
NSH = 12500          # nodes per core
NSH_PAD = 12544      # 98 * 128
NFULL = P * NSH_PAD  # 100352 rows in allgathered x
ALPHA = 0.5
CALL = 896           # idxs per call (56 idx cols per call: ring-safe)
COLS = CALL // 16
# gather chunks over x_full rows (int16 idx limit 32767): chunk c = rows
# [32768c, 32768c+32768); chunk 3 is short (2048 rows)
NCHUNK = 4
CHUNK_ROWS = (32768, 32768, 32768, NFULL - 3 * 32768)
# per-direction per-chunk call capacity (fixed program shape; ~20% margin
# over the expected rank-grouped call count for E/P random edges)
CAPS = (50, 50, 50, 8)
NC_DIR = sum(CAPS)          # calls per direction
CHUNK_BASE = (0, CAPS[0], CAPS[0] + CAPS[1], CAPS[0] + CAPS[1] + CAPS[2])

_verbose = True


def _log(tag, t0):
    if _verbose:
        now = _time.perf_counter()
        print(f"[kernel-timing] {tag}: {now - t0:.3f}s", file=sys.stderr)
    return _time.perf_counter()


# ---------------------------------------------------------------- planner ---

def _plan_direction(gat, seg, caps):
    """Vectorized duplicate-free call plan for one direction.

    gat: global gather node per edge; seg: global segment (scatter) node.
    Returns (gflat, sflat) int16 arrays [P, NC_DIR*CALL] filled with -1 pads,
    plus per-(core,chunk) needed call counts [P, NCHUNK].
    Within one call every scatter target is unique (edges grouped by
    round-robin rank within their (chunk, segment)), so no same-row RMW
    races inside a call.
    """
    seg = seg.astype(np.int32)
    gat = gat.astype(np.int32)
    core = seg // NSH
    loc = seg - core * NSH
    q = gat // NSH
    row = q * NSH_PAD + (gat - q * NSH)
    chunk = row >> 15
    lidx = row & 32767

    cc = core * NCHUNK + chunk                 # 0..31
    # sort by (core, chunk, seg); rank = run position within equal seg
    k1 = cc * (1 << 17) + seg                  # < 2^22, int32 radix sort
    o1 = np.argsort(k1, kind="stable")
    k1s = k1[o1]
    first = np.empty(E, bool)
    first[0] = True
    np.not_equal(k1s[1:], k1s[:-1], out=first[1:])
    ar = np.arange(E, dtype=np.int64)
    idx_first = np.maximum.accumulate(np.where(first, ar, 0))
    rank = (ar - idx_first).astype(np.int32)
    # sort by (core, chunk, rank) stable -> final edge order
    k2 = cc[o1] * E + rank                     # < 2^25, int32 radix sort
    o2 = np.argsort(k2, kind="stable")
    k2s = k2[o2]
    of = o1[o2]
    # position within each (core, chunk, rank) run
    first2 = np.empty(E, bool)
    first2[0] = True
    np.not_equal(k2s[1:], k2s[:-1], out=first2[1:])
    idx_first2 = np.maximum.accumulate(np.where(first2, ar, 0))
    posr = ar - idx_first2
    call_in_run = posr // CALL
    slot = posr - call_in_run * CALL
    # per-run call counts -> per-(core,chunk) cumulative call base per rank
    run_starts = np.flatnonzero(first2)
    run_lens = np.diff(np.r_[run_starts, E])
    run_calls = (run_lens + CALL - 1) // CALL
    run_cc = cc[of[run_starts]]
    # cumulative calls of earlier ranks within same (core,chunk)
    csum = np.cumsum(run_calls) - run_calls
    cc_first_run = np.empty(run_cc.size, bool)
    cc_first_run[0] = True
    np.not_equal(run_cc[1:], run_cc[:-1], out=cc_first_run[1:])
    arr_r = np.arange(run_cc.size)
    idx_first_cc = np.maximum.accumulate(np.where(cc_first_run, arr_r, 0))
    run_base = csum - csum[idx_first_cc]
    needed = np.zeros((P, NCHUNK), np.int64)
    last_of_cc = np.r_[cc_first_run[1:], True]
    needed[run_cc[last_of_cc] // NCHUNK, run_cc[last_of_cc] % NCHUNK] = \
        (run_base + run_calls)[last_of_cc]
    if np.any(needed > np.asarray(caps)[None, :]):
        return None, None, needed
    # flat destination index
    call_idx = run_base[np.cumsum(first2) - 1] + call_in_run
    ch_base = np.asarray(
        [0, caps[0], caps[0] + caps[1], caps[0] + caps[1] + caps[2]])
    nc_dir = int(sum(caps))
    core_f = core[of]
    chunk_f = chunk[of]
    flat = ((core_f * nc_dir + ch_base[chunk_f] + call_idx) * CALL + slot)
    # pads gather row 0 of their chunk and scatter into agg row NSH (a
    # discarded pad row); same-row pad adds may race but are never read
    gflat = np.zeros(P * nc_dir * CALL, np.int16)
    sflat = np.full(P * nc_dir * CALL, NSH, np.int16)
    gflat[flat] = lidx[of].astype(np.int16)
    sflat[flat] = loc[of].astype(np.int16)
    return (gflat.reshape(P, nc_dir * CALL),
            sflat.reshape(P, nc_dir * CALL), needed)


def _wrap16(a):
    # idx i -> [i % 16, i // 16]
    return np.ascontiguousarray(a.reshape(-1, 16).T)


# ------------------------------------------------------------- device prog --

def _build_program(caps):
    from concourse import bacc, tile, mybir, library_config

    f32 = mybir.dt.float32
    f16 = mybir.dt.float16
    i16 = mybir.dt.int16
    nc = bacc.Bacc("TRN2", target_bir_lowering=False, debug=False,
                   num_swdge_queues=3, num_devices=P)

    nc_dir = int(sum(caps))
    xs = nc.dram_tensor("x_sh", [NSH_PAD, D], f16, kind="ExternalInput")
    # all four idx streams packed in one tensor (rows: 16 per stream, order
    # gidx_in, sidx_in, gidx_out, sidx_out); all small f32 constants packed
    # in one [128, 709] tensor (cols: inv_in 0:98, inv_out 98:196,
    # W_self 196:324, W1 324:452, W2 452:580, b 580:581, ident 581:709)
    idx_all = nc.dram_tensor("idx_all", [64, nc_dir * COLS], i16,
                             kind="ExternalInput")
    consts = nc.dram_tensor("consts", [128, 709], f32, kind="ExternalInput")
    x_bounce = nc.dram_tensor("x_bounce", [NSH_PAD, D], f16)
    # NOTE: addr_space="Shared" for the AllGather output desyncs the mesh
    # under the axon PJRT path; plain DRAM works (slower CC but tiny anyway).
    x_full = nc.dram_tensor("x_full", [NFULL, D], f16)
    agg_in = nc.dram_tensor("agg_in", [NSH_PAD, D], f32)
    agg_out = nc.dram_tensor("agg_out", [NSH_PAD, D], f32)
    out = nc.dram_tensor("out", [NSH_PAD, D], f16, kind="ExternalOutput")

    with tile.TileContext(nc) as tc:
        nc.gpsimd.load_library(library_config.mlp)
        with tc.tile_pool(name="const", bufs=1) as cp, \
             tc.tile_pool(name="gt", bufs=2) as gp, \
             tc.tile_pool(name="ep", bufs=3) as ep, \
             tc.tile_pool(name="ps", bufs=2, space="PSUM") as pp:
            # shard -> bounce -> allgather into x_full
            nc.sync.dma_start(x_bounce[:], xs[:])
            nc.gpsimd.collective_compute(
                "AllGather", mybir.AluOpType.bypass,
                replica_groups=[list(range(P))],
                ins=[x_bounce.ap().opt()],
                outs=[x_full.ap().opt()],
            )

            # index tiles: load 16 rows, replicate to 128 partitions on device
            idx_tiles = {}
            for j, nm in enumerate(("gi", "si", "go", "so")):
                t = cp.tile([128, nc_dir * COLS], i16, tag=f"idx_{nm}")
                nc.sync.dma_start(t[0:16, :], idx_all[j * 16:(j + 1) * 16, :])
                nc.sync.dma_start(t[16:32, :], idx_all[j * 16:(j + 1) * 16, :])
                nc.sync.dma_start(t[32:64, :], t[0:32, :])
                nc.sync.dma_start(t[64:128, :], t[0:64, :])
                idx_tiles[nm] = t
            cs = cp.tile([128, 709], f32)
            nc.sync.dma_start(cs[:], consts[:])
            C_IVI, C_IVO, C_WS, C_W1, C_W2, C_B, C_ID = (
                0, 98, 196, 324, 452, 580, 581)

            # zero agg buffers from an SBUF zero tile
            zt = cp.tile([128, NSH_PAD], f32)
            nc.vector.memset(zt[:], 0.0)
            for t in range(98):
                nc.sync.dma_start(agg_in[t * 128:(t + 1) * 128, :],
                                  zt[:, t * 128:(t + 1) * 128])
                nc.sync.dma_start(agg_out[t * 128:(t + 1) * 128, :],
                                  zt[:, t * 128:(t + 1) * 128])

            # gather + convert + scatter chains; a dummy reader of the agg
            # buffer between consecutive same-buffer scatters forces each
            # scatter's DMA to complete before the next starts (cross-call
            # same-row RMWs on different DMA engines would otherwise race).
            def emit_call(k, c, gkey, skey, agg, dirtag):
                t16 = gp.tile([128, CALL // 128, D], f16, tag="g16")
                if c < 3:
                    src = x_full[c * 32768:(c + 1) * 32768, :]
                else:
                    src = x_full[3 * 32768:NFULL, :]
                nc.gpsimd.dma_gather(
                    t16[:], src,
                    idx_tiles[gkey][:, k * COLS:(k + 1) * COLS],
                    CALL, CALL, D, queue_num=0)
                t32 = gp.tile([128, CALL // 128, D], f32, tag="g32")
                nc.vector.tensor_copy(t32[:], t16[:])
                dr = gp.tile([1, 64], f32, tag=f"dummy{dirtag}")
                nc.sync.dma_start(dr[:], agg[0:1, 0:64])
                nc.gpsimd.dma_scatter_add(
                    agg[:], t32[:],
                    idx_tiles[skey][:, k * COLS:(k + 1) * COLS],
                    CALL, CALL, D, queue_num=1 if dirtag == "i" else 2)

            # interleave the 8 (direction, chunk) streams round-robin so that
            # consecutive same-buffer scatters sharing a dst row are far
            # apart in time (adjacent same-stream calls are dst-disjoint
            # splits of one rank group or consecutive ranks)
            ch_base = (0, caps[0], caps[0] + caps[1], caps[0] + caps[1] + caps[2])
            for r in range(max(caps)):
                for c in range(NCHUNK):
                    if r < caps[c]:
                        emit_call(ch_base[c] + r, c, "gi", "si", agg_in, "i")
                        emit_call(ch_base[c] + r, c, "go", "so", agg_out, "o")

            # epilogue per 128-node tile
            for t in range(98):
                ai = ep.tile([128, D], f32, tag="ai")
                ao = ep.tile([128, D], f32, tag="ao")
                nc.sync.dma_start(ai[:], agg_in[t * 128:(t + 1) * 128, :])
                nc.sync.dma_start(ao[:], agg_out[t * 128:(t + 1) * 128, :])
                # scale by inv degree (per-partition scalar)
                nc.vector.tensor_scalar(ai[:], ai[:],
                                        cs[:, C_IVI + t:C_IVI + t + 1], None,
                                        mybir.AluOpType.mult)
                nc.vector.tensor_scalar(ao[:], ao[:],
                                        cs[:, C_IVO + t:C_IVO + t + 1], None,
                                        mybir.AluOpType.mult)
                # own-shard x tile: load f16, convert to f32
                xt16 = ep.tile([128, D], f16, tag="xt16")
                nc.sync.dma_start(xt16[:], xs[t * 128:(t + 1) * 128, :])
                xt = ep.tile([128, D], f32, tag="xt")
                nc.vector.tensor_copy(xt[:], xt16[:])
                # transpose all three activations
                pt = pp.tile([128, D], f32, tag="pt")
                nc.tensor.matmul(pt[:], ai[:], cs[:, C_ID:C_ID + D],
                                 start=True, stop=True, is_transpose=True)
                aiT = ep.tile([128, D], f32, tag="aiT")
                nc.vector.tensor_copy(aiT[:], pt[:])
                pt2 = pp.tile([128, D], f32, tag="pt")
                nc.tensor.matmul(pt2[:], ao[:], cs[:, C_ID:C_ID + D],
                                 start=True, stop=True, is_transpose=True)
                aoT = ep.tile([128, D], f32, tag="aoT")
                nc.vector.tensor_copy(aoT[:], pt2[:])
                pt3 = pp.tile([128, D], f32, tag="pt")
                nc.tensor.matmul(pt3[:], xt[:], cs[:, C_ID:C_ID + D],
                                 start=True, stop=True, is_transpose=True)
                xtT = ep.tile([128, D], f32, tag="xtT")
                nc.vector.tensor_copy(xtT[:], pt3[:])
                # y = W_self.T @ xT + W1.T @ aiT + W2.T @ aoT   [feat_out, nodes]
                y = pp.tile([128, 128], f32, tag="y")
                nc.tensor.matmul(y[:], cs[:, C_WS:C_WS + D], xtT[:],
                                 start=True, stop=False)
                nc.tensor.matmul(y[:], cs[:, C_W1:C_W1 + D], aiT[:],
                                 start=False, stop=False)
                nc.tensor.matmul(y[:], cs[:, C_W2:C_W2 + D], aoT[:],
                                 start=False, stop=True)
                ysb = ep.tile([128, 128], f32, tag="ysb")
                nc.vector.tensor_scalar(ysb[:], y[:], cs[:, C_B:C_B + 1], None,
                                        mybir.AluOpType.add)
                # transpose back to [nodes, feat], convert to f16
                po = pp.tile([128, 128], f32, tag="po")
                nc.tensor.matmul(po[:], ysb[:], cs[:, C_ID:C_ID + D],
                                 start=True, stop=True, is_transpose=True)
                osb = ep.tile([128, 128], f16, tag="osb")
                nc.vector.tensor_copy(osb[:], po[:])
                nc.sync.dma_start(out[t * 128:(t + 1) * 128, :], osb[:])

    nc.compile()
    return nc


# --------------------------------------------------------------- AOT setup --

_AOT = {}


def _aot_compile(caps):
    t0 = _time.perf_counter()
    import jax
    from jax.sharding import Mesh, PartitionSpec, NamedSharding
    from jax.experimental.shard_map import shard_map
    from concourse import bass2jax, mybir

    nc = _build_program(caps)
    t0 = _log("aot: build+bass-compile", t0)

    bass2jax.install_neuronx_cc_hook()
    partition_name = nc.partition_id_tensor.name if nc.partition_id_tensor else None
    in_names, out_names, out_avals, zero_outs = [], [], [], []
    for alloc in nc.m.functions[0].allocations:
        if not isinstance(alloc, mybir.MemoryLocationSet):
            continue
        name = alloc.memorylocations[0].name
        if alloc.kind == "ExternalInput":
            if name != partition_name:
                in_names.append(name)
        elif alloc.kind == "ExternalOutput":
            shape = tuple(alloc.tensor_shape)
            dtype = mybir.dt.np(alloc.dtype)
            out_names.append(name)
            out_avals.append(jax.core.ShapedArray(shape, dtype))
            zero_outs.append(np.zeros(shape, dtype))
    n_params = len(in_names)
    n_outs = len(out_avals)
    in_names_full = in_names + out_names + ([partition_name] if partition_name else [])

    def _body(*args):
        operands = list(args)
        if partition_name is not None:
            operands.append(bass2jax.partition_id_tensor())
        outs = bass2jax._bass_exec_p.bind(
            *operands,
            out_avals=tuple(out_avals),
            in_names=tuple(in_names_full),
            out_names=tuple(out_names),
            lowering_input_output_aliases=(),
            sim_require_finite=True,
            sim_require_nnan=True,
            nc=nc,
        )
        return tuple(outs)

    devices = jax.devices()[:P]
    mesh = Mesh(np.asarray(devices), ("core",))
    in_specs = (PartitionSpec("core"),) * (n_params + n_outs)
    out_specs = (PartitionSpec("core"),) * n_outs
    donate = tuple(range(n_params, n_params + n_outs))
    sharded = jax.jit(
        shard_map(_body, mesh=mesh, in_specs=in_specs, out_specs=out_specs,
                  check_rep=False),
        donate_argnums=donate, keep_unused=True)

    # abstract shapes: per-core input shapes concatenated over cores on axis 0
    shape_of = {}
    for alloc in nc.m.functions[0].allocations:
        if isinstance(alloc, mybir.MemoryLocationSet) and alloc.kind == "ExternalInput":
            shape_of[alloc.memorylocations[0].name] = (
                tuple(alloc.tensor_shape), mybir.dt.np(alloc.dtype))
    abstract = []
    for name in in_names:
        shp, dt = shape_of[name]
        abstract.append(jax.ShapeDtypeStruct((P * shp[0],) + shp[1:], dt))
    for z in zero_outs:
        abstract.append(jax.ShapeDtypeStruct((P * z.shape[0],) + z.shape[1:], z.dtype))
    compiled = sharded.lower(*abstract).compile()
    t0 = _log("aot: neff-compile", t0)

    sharding = NamedSharding(mesh, PartitionSpec("core"))
    _AOT.update(dict(
        caps=tuple(caps), nc=nc, compiled=compiled, in_names=in_names,
        out_names=out_names, zero_outs=zero_outs, sharding=sharding,
        dz=None, jax=jax))
    _make_zeros()
    _log("aot: zero-put", t0)


def _make_zeros():
    jax = _AOT["jax"]
    _AOT["dz"] = [
        jax.device_put(
            np.zeros((P * z.shape[0],) + z.shape[1:], z.dtype), _AOT["sharding"])
        for z in _AOT["zero_outs"]]
    jax.block_until_ready(_AOT["dz"])


import os
if not os.environ.get("KERNEL_NO_AOT"):
    try:
        _aot_compile(CAPS)
    except Exception as _e:  # pragma: no cover - fall through to lazy compile
        print(f"[kernel] AOT compile failed ({_e!r}); will compile lazily",
              file=sys.stderr)
        _AOT.clear()


# ------------------------------------------------------------------ kernel --

def kernel(x, W_self, b_self, W_s2d, b_s2d, W_d2s, b_d2s, edge_index):
    t0 = _time.perf_counter()
    x = np.ascontiguousarray(x, np.float32)
    W_self = np.asarray(W_self, np.float32)
    b_self = np.asarray(b_self, np.float32)
    W_s2d = np.asarray(W_s2d, np.float32)
    b_s2d = np.asarray(b_s2d, np.float32)
    W_d2s = np.asarray(W_d2s, np.float32)
    b_d2s = np.asarray(b_d2s, np.float32)
    src = np.asarray(edge_index[0], np.int64)
    dst = np.asarray(edge_index[1], np.int64)

    jax = _AOT.get("jax")
    if jax is None:
        import jax  # lazy path
    sharding = _AOT.get("sharding")

    # x conversion first (cheap, serial), then planner threads overlap with
    # the x upload and the consts packing on the main thread
    from concurrent.futures import ThreadPoolExecutor
    caps = _AOT.get("caps", CAPS)
    nc_dir = int(sum(caps))
    x16 = np.zeros((NFULL, D), np.float16)
    x16.reshape(P, NSH_PAD, D)[:, :NSH] = x.reshape(P, NSH, D).astype(np.float16)

    def _pack_idx(v, out):
        # idx i -> [i % 16, i // 16] per core
        out[:] = v.reshape(P, nc_dir * COLS, 16).transpose(0, 2, 1)

    def _build_consts():
        deg_in = np.bincount(dst, minlength=N).astype(np.float32)
        deg_out = np.bincount(src, minlength=N).astype(np.float32)
        con = np.zeros((P, 128, 709), np.float32)
        for col, v in ((0, 1.0 / np.maximum(deg_in, 1.0)),
                       (98, 1.0 / np.maximum(deg_out, 1.0))):
            a = np.zeros((P, NSH_PAD), np.float32)
            a[:, :NSH] = v.reshape(P, NSH)
            con[:, :, col:col + 98] = a.reshape(P, 98, 128).transpose(0, 2, 1)
        con[:, :, 196:324] = W_self
        con[:, :, 324:452] = (1.0 - ALPHA) * W_s2d
        con[:, :, 452:580] = ALPHA * W_d2s
        b_tot = b_self + (1.0 - ALPHA) * b_s2d + ALPHA * b_d2s
        con[:, :, 580] = b_tot
        con[:, :, 581:709] = np.eye(D, dtype=np.float32)
        return con.reshape(P * 128, 709)

    def _plan_all(caps_, idx_host, pool):
        f_in = pool.submit(_plan_direction, src, dst, caps_)
        f_out = pool.submit(_plan_direction, dst, src, caps_)
        gi, si, need_i = f_in.result()
        go, so, need_o = f_out.result()
        if gi is None or go is None:
            return np.maximum(need_i, need_o)
        nd = int(sum(caps_))
        for j, arr in enumerate((gi, si, go, so)):
            idx_host.reshape(P, 64, nd * COLS)[:, j * 16:(j + 1) * 16] = \
                arr.reshape(P, nd * COLS, 16).transpose(0, 2, 1)
        return None

    with ThreadPoolExecutor(3) as ex:
        idx_host = np.empty((P * 64, nc_dir * COLS), np.int16)
        f_plan = ex.submit(_plan_all, caps, idx_host, ex)
        # main: x upload + consts pack/upload overlap with planning
        dev_x = jax.device_put(x16, sharding) if sharding is not None else None
        con = _build_consts()
        dev_con = (jax.device_put(con, sharding)
                   if sharding is not None else None)
        need = f_plan.result()

    if need is not None:
        # capacity exceeded: recompile with room and redo plans
        caps = tuple(int(v)
                     for v in np.maximum(np.asarray(caps), need.max(0) + 2))
        print(f"[kernel] capacity exceeded; recompiling with caps={caps}",
              file=sys.stderr)
        _aot_compile(caps)
        sharding = _AOT["sharding"]
        jax = _AOT["jax"]
        nc_dir = int(sum(caps))
        dev_x = jax.device_put(x16, sharding)
        dev_con = jax.device_put(con, sharding)
        idx_host = np.empty((P * 64, nc_dir * COLS), np.int16)
        with ThreadPoolExecutor(3) as ex:
            assert _plan_all(caps, idx_host, ex) is None
    t0 = _log("plan+pack+put", t0)

    if not _AOT:
        _aot_compile(caps)
        jax = _AOT["jax"]
        sharding = _AOT["sharding"]
        dev_x = jax.device_put(x16, sharding)
        dev_con = jax.device_put(con, sharding)
    compiled = _AOT["compiled"]
    dev_idx = jax.device_put(idx_host, sharding)

    by_name = {"x_sh": dev_x, "idx_all": dev_idx, "consts": dev_con}
    dev_args = [by_name[name] for name in _AOT["in_names"]]
    if _AOT["dz"] is None:
        _make_zeros()
    dz = _AOT["dz"]
    _AOT["dz"] = None  # consumed by donation
    jax.block_until_ready(dev_args)
    t0 = _log("upload", t0)

    try:
        outs = compiled(*dev_args, *dz)
        jax.block_until_ready(outs)
    except Exception as e:  # transient device failure: one retry
        print(f"[kernel] exec failed ({e!r}); retrying once", file=sys.stderr)
        _make_zeros()
        dz = _AOT["dz"]
        _AOT["dz"] = None
        outs = compiled(*dev_args, *dz)
        jax.block_until_ready(outs)
    t0 = _log("exec", t0)

    # fetch the 8 output shards in parallel and convert per shard
    res = np.empty((N, D), np.float32)
    res_v = res.reshape(P, NSH, D)
    shards = list(outs[0].addressable_shards)

    def _fetch(sh):
        c = sh.index[0].start // NSH_PAD if sh.index[0].start else 0
        res_v[c] = np.asarray(sh.data)[:NSH].astype(np.float32)

    with ThreadPoolExecutor(P) as ex:
        list(ex.map(_fetch, shards))
    _log("fetch+convert", t0)
    return res
